# revision 11
# baseline (speedup 1.0000x reference)
"""Trainium2 kernel for ChunkLayer boundary-token compaction.

Problem: hidden_states [B=4, L=4096, D=2048] f32, boundary_mask [B, L] bool.
Per sequence, compact rows where boundary_mask is True to the front (stable
order), truncate to K = max per-sequence count (padding past a sequence's own
count comes from the earliest dropped rows, matching the reference argsort),
and emit next_mask[b, j] = j < num_tokens[b].

Strategy: the index computation is O(B*L) bool work -> host. The memory-bound
part (gathering ~B*K rows of 8 KiB each, ~66 MiB read + 66 MiB write) runs on
8 NeuronCores: core (b, h) gathers half of sequence b's K output rows with
SWDGE dma_gather chunks (HBM -> SBUF, one 8 KiB descriptor per row), then
writes them back with HWDGE stores (SBUF -> HBM, 8 KiB descriptors).
SDMA strictly prioritizes the SWDGE queue, so the two phases are effectively
serial; each phase runs at ~425 GB/s. The GPSIMD library load (~9 us, needed
for dma_gather ucode) is spliced before the kernel's all-engine barrier so it
hides under the other engines' preamble instead of the critical path.
"""

import numpy as np

import concourse.bacc as bacc
import concourse.mybir as mybir
from concourse import library_config
from concourse.bass_utils import run_bass_kernel_spmd

B, L, D = 4, 4096, 2048
N_CORES = 8
HALVES = 2  # cores per sequence
CHUNK = 512  # max rows per dma_gather (hardware-validated limit is ~1024)

# Stash of the most recent device-run results (exec_time_ns etc.) for test.py.
LAST_RESULT = None

_PROGRAM_CACHE = {}


def _round_up(x, m):
    return (x + m - 1) // m * m


def _chunks_for(num_idxs):
    """Chunk layout (row_start, rows): smallest chunk first for fast pipeline
    ramp; all sizes multiples of 128."""
    rem = num_idxs % CHUNK
    sizes = ([rem] if rem else []) + [CHUNK] * (num_idxs // CHUNK)
    out, s = [], 0
    for sz in sizes:
        out.append((s, sz))
        s += sz
    return out


def _build_program(num_idxs, valid):
    """One SPMD program: gather `valid` rows (of `num_idxs` index slots; the
    tail is -1-padded and skipped by the hardware) of D f32 from x by idx,
    store them to y (row j of y = gathered row j). Same NEFF on all 8 cores;
    per-core behavior comes entirely from the inputs."""
    assert num_idxs % 128 == 0 and 0 < valid <= num_idxs
    icols = num_idxs // 16
    chunks = _chunks_for(num_idxs)
    nbt = num_idxs // 128

    nc = bacc.Bacc("TRN2", debug=False)

    # Hoist the GPSIMD library load (dma_gather ucode, ~9 us) to before the
    # kernel's opening all-engine barrier: Q7 loads it while the other
    # engines run their preambles, instead of serializing after the barrier.
    lib_inst = nc.gpsimd.load_library(library_config.mlp)
    entry = nc.main_func.blocks[0]
    entry.instructions.remove(lib_inst.ins)
    entry.instructions.insert(
        entry.instructions.index(nc.gpsimd.preamble_end) + 1, lib_inst.ins
    )

    x = nc.dram_tensor("x", [L, D], mybir.dt.float32, kind="ExternalInput")
    idx = nc.dram_tensor("idx", [128, icols], mybir.dt.int16, kind="ExternalInput")
    y = nc.dram_tensor("y", [num_idxs, D], mybir.dt.float32, kind="ExternalOutput")

    # Per-chunk schedule: valid rows, full store blocks, partial partitions,
    # cumulative gather/store counts for semaphore bookkeeping.
    infos = []
    g_cum = s_cum = 0
    for start, rows in chunks:
        v = max(0, min(valid - start, rows))
        vb, vp = v // 128, v % 128
        if v:
            g_cum += 1
            s_cum += (1 if vb else 0) + (1 if vp else 0)
        infos.append((start, rows, v, vb, vp, g_cum, s_cum))
    total_stores = s_cum

    with (
        nc.Block() as block,
        nc.sbuf_tensor("dst", [128, nbt, D], mybir.dt.float32) as dst,
        nc.sbuf_tensor("idxs_sbuf", [128, icols], mybir.dt.int16) as idxs_sbuf,
        nc.semaphore("isem") as isem,
        nc.semaphore("gsem") as gsem,
        nc.semaphore("ssem") as ssem,
    ):
        y_view = y.rearrange("(c p) d -> p c d", p=128)  # [128, nbt, D]

        @block.gpsimd
        def _(gpsimd):
            gpsimd.wait_ge(isem, 16)
            for start, rows, v, vb, vp, g_cum, s_cum in infos:
                if v == 0:
                    continue
                c0 = start // 128
                nb = rows // 128
                gpsimd.dma_gather(
                    dst[:, c0 : c0 + nb, :],
                    x[:],
                    idxs_sbuf[:, start // 16 : (start + rows) // 16],
                    rows,
                    v,
                    D,
                ).then_inc(gsem, 16)

        @block.sync
        def _(sync):
            sync.dma_start(idxs_sbuf[:], idx[:]).then_inc(isem, 16)
            for start, rows, v, vb, vp, g_cum, s_cum in infos:
                if v == 0:
                    continue
                sync.wait_ge(gsem, 16 * g_cum)
                c0 = start // 128
                if vb:
                    sync.dma_start(
                        y_view[:, c0 : c0 + vb, :], dst[:, c0 : c0 + vb, :]
                    ).then_inc(ssem, 16)
                if vp:
                    sync.dma_start(
                        y_view[:vp, c0 + vb, :], dst[:vp, c0 + vb, :]
                    ).then_inc(ssem, 16)
            sync.wait_ge(ssem, 16 * total_stores)

    nc.compile()
    return nc


def _wrap_idxs(idx_rows, num_idxs):
    """Pack a [num_idxs] int array into the dma_gather index layout:
    [128, num_idxs//16] int16, idx j at partition j%16 column j//16,
    replicated across the 8 GpSimd core groups of 16 partitions."""
    arr = np.asarray(idx_rows, dtype=np.int16).reshape(num_idxs // 16, 16).T
    return np.ascontiguousarray(np.tile(arr, (8, 1)))


def kernel(hidden_states, boundary_mask, mask):
    global LAST_RESULT
    hs = np.ascontiguousarray(np.asarray(hidden_states, dtype=np.float32))
    bm = np.asarray(boundary_mask).astype(bool)
    assert hs.shape == (B, L, D) and bm.shape == (B, L)

    counts = bm.sum(axis=1)
    K = int(counts.max())
    half = (K + HALVES - 1) // HALVES  # rows per core; h=1 may have one fewer
    num_idxs = _round_up(max(half, 128), 128)

    # Per-sequence gather indices: kept positions first, then the earliest
    # dropped positions to fill up to K (matches the reference's stable
    # argsort of arange(L) + (~mask)*L, truncated to K).
    in_maps = []
    for b in range(B):
        kept = np.flatnonzero(bm[b])
        if kept.size < K:
            dropped = np.flatnonzero(~bm[b])[: K - kept.size]
            rows = np.concatenate([kept, dropped])
        else:
            rows = kept[:K]
        for h in range(HALVES):
            shard = rows[h * half : min((h + 1) * half, K)].astype(np.int64)
            if shard.size < half:  # h=1 when K is odd: one extra benign row
                shard = np.concatenate(
                    [shard, np.zeros(half - shard.size, dtype=np.int64)]
                )
            pad = np.full(num_idxs - shard.size, -1, dtype=np.int64)
            in_maps.append(
                {
                    "x": hs[b],
                    "idx": _wrap_idxs(np.concatenate([shard, pad]), num_idxs),
                }
            )

    key = (num_idxs, half)
    if key not in _PROGRAM_CACHE:
        _PROGRAM_CACHE[key] = _build_program(num_idxs, half)
    nc = _PROGRAM_CACHE[key]

    LAST_RESULT = run_bass_kernel_spmd(nc, in_maps, core_ids=list(range(N_CORES)))

    next_hidden_states = np.empty((B, K, D), dtype=np.float32)
    for b in range(B):
        for h in range(HALVES):
            rows_out = LAST_RESULT.results[HALVES * b + h]["y"]
            lo = h * half
            hi = min(lo + half, K)
            next_hidden_states[b, lo:hi] = rows_out[: hi - lo]
    next_mask = np.arange(K)[None, :] < counts[:, None]
    return next_hidden_states, next_mask


# revision 13
# speedup vs baseline: 1.0212x; 1.0212x over previous
"""Trainium2 kernel for ChunkLayer boundary-token compaction.

Problem: hidden_states [B=4, L=4096, D=2048] f32, boundary_mask [B, L] bool.
Per sequence, compact rows where boundary_mask is True to the front (stable
order), truncate to K = max per-sequence count (padding past a sequence's own
count comes from the earliest dropped rows, matching the reference argsort),
and emit next_mask[b, j] = j < num_tokens[b].

Strategy: the index computation is O(B*L) bool work -> host. The memory-bound
part (gathering ~B*K rows of 8 KiB each, ~66 MiB read + 66 MiB write) runs on
8 NeuronCores: core (b, h) gathers half of sequence b's K output rows with
SWDGE dma_gather chunks (HBM -> SBUF, one 8 KiB descriptor per row), then
writes them back with HWDGE stores (SBUF -> HBM, 8 KiB descriptors).
SDMA strictly prioritizes the SWDGE queue, so the two phases are effectively
serial; each phase runs at ~425 GB/s. The GPSIMD library load (~9 us, needed
for dma_gather ucode) is spliced before the kernel's all-engine barrier so it
hides under the other engines' preamble instead of the critical path.
"""

import numpy as np

import concourse.bacc as bacc
import concourse.mybir as mybir
from concourse import library_config
from concourse.bass_utils import run_bass_kernel_spmd

B, L, D = 4, 4096, 2048
N_CORES = 8
HALVES = 2  # cores per sequence
CHUNK = 512  # max rows per dma_gather (hardware-validated limit is ~1024)

# Stash of the most recent device-run results (exec_time_ns etc.) for test.py.
LAST_RESULT = None

_PROGRAM_CACHE = {}


def _round_up(x, m):
    return (x + m - 1) // m * m


def _chunks_for(num_idxs):
    """Chunk layout (row_start, rows): smallest chunk first for fast pipeline
    ramp; all sizes multiples of 128."""
    rem = num_idxs % CHUNK
    sizes = ([rem] if rem else []) + [CHUNK] * (num_idxs // CHUNK)
    out, s = [], 0
    for sz in sizes:
        out.append((s, sz))
        s += sz
    return out


def _build_program(num_idxs, valid):
    """One SPMD program: gather `valid` rows (of `num_idxs` index slots; the
    tail is -1-padded and skipped by the hardware) of D f32 from x by idx,
    store them to y (row j of y = gathered row j). Same NEFF on all 8 cores;
    per-core behavior comes entirely from the inputs."""
    assert num_idxs % 128 == 0 and 0 < valid <= num_idxs
    icols = num_idxs // 16
    chunks = _chunks_for(num_idxs)
    nbt = num_idxs // 128

    nc = bacc.Bacc("TRN2", debug=False)
    x = nc.dram_tensor("x", [L, D], mybir.dt.float32, kind="ExternalInput")
    idx = nc.dram_tensor("idx", [128, icols], mybir.dt.int16, kind="ExternalInput")
    y = nc.dram_tensor("y", [num_idxs, D], mybir.dt.float32, kind="ExternalOutput")

    # Per-chunk schedule: valid rows, full store blocks, partial partitions,
    # cumulative gather/store counts for semaphore bookkeeping.
    infos = []
    g_cum = s_cum = 0
    for start, rows in chunks:
        v = max(0, min(valid - start, rows))
        vb, vp = v // 128, v % 128
        if v:
            g_cum += 1
            s_cum += (1 if vb else 0) + (1 if vp else 0)
        infos.append((start, rows, v, vb, vp, g_cum, s_cum))
    total_stores = s_cum

    with (
        nc.Block() as block,
        nc.sbuf_tensor("dst", [128, nbt, D], mybir.dt.float32) as dst,
        nc.sbuf_tensor("idxs_sbuf", [128, icols], mybir.dt.int16) as idxs_sbuf,
        nc.semaphore("isem") as isem,
        nc.semaphore("gsem") as gsem,
        nc.semaphore("ssem") as ssem,
    ):
        y_view = y.rearrange("(c p) d -> p c d", p=128)  # [128, nbt, D]

        @block.gpsimd
        def _(gpsimd):
            # overlaps the idx DMA issued from sync below
            gpsimd.load_library(library_config.mlp)
            gpsimd.wait_ge(isem, 16)
            for start, rows, v, vb, vp, g_cum, s_cum in infos:
                if v == 0:
                    continue
                c0 = start // 128
                nb = rows // 128
                gpsimd.dma_gather(
                    dst[:, c0 : c0 + nb, :],
                    x[:],
                    idxs_sbuf[:, start // 16 : (start + rows) // 16],
                    rows,
                    v,
                    D,
                ).then_inc(gsem, 16)

        @block.sync
        def _(sync):
            sync.dma_start(idxs_sbuf[:], idx[:]).then_inc(isem, 16)
            for start, rows, v, vb, vp, g_cum, s_cum in infos:
                if v == 0:
                    continue
                sync.wait_ge(gsem, 16 * g_cum)
                c0 = start // 128
                if vb:
                    sync.dma_start(
                        y_view[:, c0 : c0 + vb, :], dst[:, c0 : c0 + vb, :]
                    ).then_inc(ssem, 16)
                if vp:
                    sync.dma_start(
                        y_view[:vp, c0 + vb, :], dst[:vp, c0 + vb, :]
                    ).then_inc(ssem, 16)
            sync.wait_ge(ssem, 16 * total_stores)

    nc.compile()
    return nc


def _wrap_idxs(idx_rows, num_idxs):
    """Pack a [num_idxs] int array into the dma_gather index layout:
    [128, num_idxs//16] int16, idx j at partition j%16 column j//16,
    replicated across the 8 GpSimd core groups of 16 partitions."""
    arr = np.asarray(idx_rows, dtype=np.int16).reshape(num_idxs // 16, 16).T
    return np.ascontiguousarray(np.tile(arr, (8, 1)))


def kernel(hidden_states, boundary_mask, mask):
    global LAST_RESULT
    hs = np.ascontiguousarray(np.asarray(hidden_states, dtype=np.float32))
    bm = np.asarray(boundary_mask).astype(bool)
    assert hs.shape == (B, L, D) and bm.shape == (B, L)

    counts = bm.sum(axis=1)
    K = int(counts.max())
    half = (K + HALVES - 1) // HALVES  # rows per core; h=1 may have one fewer
    num_idxs = _round_up(max(half, 128), 128)

    # Per-sequence gather indices: kept positions first, then the earliest
    # dropped positions to fill up to K (matches the reference's stable
    # argsort of arange(L) + (~mask)*L, truncated to K).
    in_maps = []
    for b in range(B):
        kept = np.flatnonzero(bm[b])
        if kept.size < K:
            dropped = np.flatnonzero(~bm[b])[: K - kept.size]
            rows = np.concatenate([kept, dropped])
        else:
            rows = kept[:K]
        for h in range(HALVES):
            shard = rows[h * half : min((h + 1) * half, K)].astype(np.int64)
            if shard.size < half:  # h=1 when K is odd: one extra benign row
                shard = np.concatenate(
                    [shard, np.zeros(half - shard.size, dtype=np.int64)]
                )
            pad = np.full(num_idxs - shard.size, -1, dtype=np.int64)
            in_maps.append(
                {
                    "x": hs[b],
                    "idx": _wrap_idxs(np.concatenate([shard, pad]), num_idxs),
                }
            )

    key = (num_idxs, half)
    if key not in _PROGRAM_CACHE:
        _PROGRAM_CACHE[key] = _build_program(num_idxs, half)
    nc = _PROGRAM_CACHE[key]

    LAST_RESULT = run_bass_kernel_spmd(nc, in_maps, core_ids=list(range(N_CORES)))

    next_hidden_states = np.empty((B, K, D), dtype=np.float32)
    for b in range(B):
        for h in range(HALVES):
            rows_out = LAST_RESULT.results[HALVES * b + h]["y"]
            lo = h * half
            hi = min(lo + half, K)
            next_hidden_states[b, lo:hi] = rows_out[: hi - lo]
    next_mask = np.arange(K)[None, :] < counts[:, None]
    return next_hidden_states, next_mask


# revision 14
# speedup vs baseline: 1.1170x; 1.0939x over previous
"""Trainium2 kernel for ChunkLayer boundary-token compaction.

Problem: hidden_states [B=4, L=4096, D=2048] f32, boundary_mask [B, L] bool.
Per sequence, compact rows where boundary_mask is True to the front (stable
order), truncate to K = max per-sequence count (padding past a sequence's own
count comes from the earliest dropped rows, matching the reference argsort),
and emit next_mask[b, j] = j < num_tokens[b].

Strategy: the index computation is O(B*L) bool work -> host. The memory-bound
part (gathering ~B*K rows of 8 KiB each, ~66 MiB read + 66 MiB write) runs on
8 NeuronCores: core (b, h) gathers half of sequence b's K output rows with
SWDGE dma_gather chunks (HBM -> SBUF, one 8 KiB descriptor per row), then
writes them back with HWDGE stores (SBUF -> HBM, 8 KiB descriptors).
SDMA strictly prioritizes the SWDGE queue, so the two phases are effectively
serial; each phase runs at ~425 GB/s. The GPSIMD library load (~9 us, needed
for dma_gather ucode) is spliced before the kernel's all-engine barrier so it
hides under the other engines' preamble instead of the critical path.
"""

import numpy as np

import concourse.bacc as bacc
import concourse.mybir as mybir
from concourse import library_config
from concourse.bass_utils import run_bass_kernel_spmd

B, L, D = 4, 4096, 2048
N_CORES = 8
HALVES = 2  # cores per sequence
CHUNK = 512  # max rows per dma_gather (hardware-validated limit is ~1024)

# Stash of the most recent device-run results (exec_time_ns etc.) for test.py.
LAST_RESULT = None

_PROGRAM_CACHE = {}


def _round_up(x, m):
    return (x + m - 1) // m * m


def _chunks_for(num_idxs):
    """Chunk layout (row_start, rows): smallest chunk first for fast pipeline
    ramp; all sizes multiples of 128."""
    rem = num_idxs % CHUNK
    sizes = ([rem] if rem else []) + [CHUNK] * (num_idxs // CHUNK)
    out, s = [], 0
    for sz in sizes:
        out.append((s, sz))
        s += sz
    return out


def _build_program(num_idxs, valid):
    """One SPMD program: gather `valid` rows (of `num_idxs` index slots; the
    tail is -1-padded and skipped by the hardware) of D f32 from x by idx,
    store them to y (row j of y = gathered row j). Same NEFF on all 8 cores;
    per-core behavior comes entirely from the inputs."""
    assert num_idxs % 128 == 0 and 0 < valid <= num_idxs
    icols = num_idxs // 16
    chunks = _chunks_for(num_idxs)
    nbt = num_idxs // 128

    nc = bacc.Bacc("TRN2", debug=False, dynamic_dma_scratch_size=65536)
    x = nc.dram_tensor("x", [L, D], mybir.dt.float32, kind="ExternalInput")
    idx = nc.dram_tensor("idx", [128, icols], mybir.dt.int16, kind="ExternalInput")
    y = nc.dram_tensor("y", [num_idxs, D], mybir.dt.float32, kind="ExternalOutput")

    # Per-chunk schedule: valid rows, full store blocks, partial partitions,
    # cumulative gather/store counts for semaphore bookkeeping.
    infos = []
    g_cum = s_cum = 0
    for start, rows in chunks:
        v = max(0, min(valid - start, rows))
        vb, vp = v // 128, v % 128
        if v:
            g_cum += 1
            s_cum += (1 if vb else 0) + (1 if vp else 0)
        infos.append((start, rows, v, vb, vp, g_cum, s_cum))
    total_stores = s_cum

    with (
        nc.Block() as block,
        nc.sbuf_tensor("dst", [128, nbt, D], mybir.dt.float32) as dst,
        nc.sbuf_tensor("idxs_sbuf", [128, icols], mybir.dt.int16) as idxs_sbuf,
        nc.semaphore("isem") as isem,
        nc.semaphore("gsem") as gsem,
        nc.semaphore("ssem") as ssem,
    ):
        y_view = y.rearrange("(c p) d -> p c d", p=128)  # [128, nbt, D]

        @block.gpsimd
        def _(gpsimd):
            # overlaps the idx DMA issued from sync below
            gpsimd.load_library(library_config.mlp)
            gpsimd.wait_ge(isem, 16)
            for start, rows, v, vb, vp, g_cum, s_cum in infos:
                if v == 0:
                    continue
                c0 = start // 128
                nb = rows // 128
                gpsimd.dma_gather(
                    dst[:, c0 : c0 + nb, :],
                    x[:],
                    idxs_sbuf[:, start // 16 : (start + rows) // 16],
                    rows,
                    v,
                    D,
                ).then_inc(gsem, 16)

        @block.sync
        def _(sync):
            sync.dma_start(idxs_sbuf[:], idx[:]).then_inc(isem, 16)
            for start, rows, v, vb, vp, g_cum, s_cum in infos:
                if v == 0:
                    continue
                sync.wait_ge(gsem, 16 * g_cum)
                c0 = start // 128
                if vb:
                    sync.dma_start(
                        y_view[:, c0 : c0 + vb, :], dst[:, c0 : c0 + vb, :]
                    ).then_inc(ssem, 16)
                if vp:
                    sync.dma_start(
                        y_view[:vp, c0 + vb, :], dst[:vp, c0 + vb, :]
                    ).then_inc(ssem, 16)
            sync.wait_ge(ssem, 16 * total_stores)

    nc.compile()
    return nc


def _wrap_idxs(idx_rows, num_idxs):
    """Pack a [num_idxs] int array into the dma_gather index layout:
    [128, num_idxs//16] int16, idx j at partition j%16 column j//16,
    replicated across the 8 GpSimd core groups of 16 partitions."""
    arr = np.asarray(idx_rows, dtype=np.int16).reshape(num_idxs // 16, 16).T
    return np.ascontiguousarray(np.tile(arr, (8, 1)))


def kernel(hidden_states, boundary_mask, mask):
    global LAST_RESULT
    hs = np.ascontiguousarray(np.asarray(hidden_states, dtype=np.float32))
    bm = np.asarray(boundary_mask).astype(bool)
    assert hs.shape == (B, L, D) and bm.shape == (B, L)

    counts = bm.sum(axis=1)
    K = int(counts.max())
    half = (K + HALVES - 1) // HALVES  # rows per core; h=1 may have one fewer
    num_idxs = _round_up(max(half, 128), 128)

    # Per-sequence gather indices: kept positions first, then the earliest
    # dropped positions to fill up to K (matches the reference's stable
    # argsort of arange(L) + (~mask)*L, truncated to K).
    in_maps = []
    for b in range(B):
        kept = np.flatnonzero(bm[b])
        if kept.size < K:
            dropped = np.flatnonzero(~bm[b])[: K - kept.size]
            rows = np.concatenate([kept, dropped])
        else:
            rows = kept[:K]
        for h in range(HALVES):
            shard = rows[h * half : min((h + 1) * half, K)].astype(np.int64)
            if shard.size < half:  # h=1 when K is odd: one extra benign row
                shard = np.concatenate(
                    [shard, np.zeros(half - shard.size, dtype=np.int64)]
                )
            pad = np.full(num_idxs - shard.size, -1, dtype=np.int64)
            in_maps.append(
                {
                    "x": hs[b],
                    "idx": _wrap_idxs(np.concatenate([shard, pad]), num_idxs),
                }
            )

    key = (num_idxs, half)
    if key not in _PROGRAM_CACHE:
        _PROGRAM_CACHE[key] = _build_program(num_idxs, half)
    nc = _PROGRAM_CACHE[key]

    LAST_RESULT = run_bass_kernel_spmd(nc, in_maps, core_ids=list(range(N_CORES)))

    next_hidden_states = np.empty((B, K, D), dtype=np.float32)
    for b in range(B):
        for h in range(HALVES):
            rows_out = LAST_RESULT.results[HALVES * b + h]["y"]
            lo = h * half
            hi = min(lo + half, K)
            next_hidden_states[b, lo:hi] = rows_out[: hi - lo]
    next_mask = np.arange(K)[None, :] < counts[:, None]
    return next_hidden_states, next_mask
